# revision 17
# baseline (speedup 1.0000x reference)
"""Trainium2 kernel for nn_BinaryDiffRow.

Math: y = x @ base_t + (x * coeff) @ S,  S = unpack_signs(mask) in {-1,+1}
Fold: y = x @ W_eff,  W_eff = base_t + coeff[:,None] * S   (single matmul)
      W_eff = (base_t - coeff) + 2*coeff*bit,  bit in {0,1}
      (base_t - coeff folded on host; bit unpacked on device)

Sharding (tensor parallel over output columns, 8 cores):
  core j owns output columns [512j, 512j+512).
  - Builds its W_eff slab (4096 x 512, bf16) once on-device:
    bit-unpack of mask via DVE shift/AND, kept resident in SBUF.
  - Streams all 8192 tokens of x (host-pretransposed, bf16) through the PE,
    accumulating psum[128tok, 512] over 32 k-chunks.
  - Host concatenates the 8 column slabs into the full output.
"""

import os
import sys

import numpy as np

for _p in ("/opt/trn_rl_repo",):
    if _p not in sys.path and os.path.isdir(_p):
        sys.path.insert(0, _p)

import ml_dtypes  # noqa: E402

# --- problem constants (hardcoded per contract) ---
B, S, IN, OUT = 4, 2048, 4096, 4096
NTOK = B * S  # 8192
NCORES = 8
OUT_SH = OUT // NCORES  # 512
P = 128
NBITS = 32

# fused tensor_scalar (int AND then float mult) passes CoreSim but the bir
# verifier rejects mixed bitwise/arith op pairs — use the unfused fallback.
FUSED_TS = False


def build_bass(in_dim=IN, ntok=NTOK, out_sh=OUT_SH, x_bufs=4, ps_bufs=4):
    """Build the single-core Bass program (SPMD: all cores run this)."""
    import concourse.mybir as mybir
    import concourse.tile as tile
    from concourse import bacc
    from contextlib import ExitStack

    kc = in_dim // P  # k-chunks
    tt = ntok // P  # token tiles
    nwords = out_sh // NBITS

    # Bacc (not plain Bass): its finalize() runs generate_event_semaphores,
    # which splits multi-sem waits — walrus only allows 1 wait/instruction.
    nc = bacc.Bacc("TRN2")
    dt = mybir.dt
    Alu = mybir.AluOpType

    xt = nc.dram_tensor("xt", (tt, P, kc, P), dt.bfloat16, kind="ExternalInput")
    # host ships (base_t - coeff) pre-tiled to (P, kc, out_sh) in bf16;
    # DMA'd directly into the resident W slab, then the unpacked +/-2c*bit
    # delta is accumulated in place (no per-k DMAs -> no DMA-wait pileups).
    bmc = nc.dram_tensor("bmc", (P, kc, out_sh), dt.bfloat16, kind="ExternalInput")
    # merged int32 const block: [shift table | mask tiled | 2*coeff bits]
    # one DMA -> one semaphore wait for all phase-1 consumers (the 3D-AP
    # TensorTensor encoding only has room for a single sync wait).
    cw = out_sh + kc * nwords + kc
    consts = nc.dram_tensor("consts", (P, cw), dt.int32, kind="ExternalInput")
    y = nc.dram_tensor("y", (ntok, out_sh), dt.float32, kind="ExternalOutput")

    with ExitStack() as ctx:
        tc = ctx.enter_context(tile.TileContext(nc))
        cpool = ctx.enter_context(tc.tile_pool(name="consts", bufs=1))
        wpool = ctx.enter_context(tc.tile_pool(name="w", bufs=1))
        upool = ctx.enter_context(tc.tile_pool(name="unpack", bufs=2))
        xpool = ctx.enter_context(tc.tile_pool(name="x", bufs=x_bufs))
        opool = ctx.enter_context(tc.tile_pool(name="out", bufs=3))
        pspool = ctx.enter_context(tc.tile_pool(name="ps", bufs=ps_bufs, space="PSUM"))

        consts_sb = cpool.tile([P, cw], dt.int32)
        nc.sync.dma_start(consts_sb[:], consts[:, :])
        shifts_sb = consts_sb[:, :out_sh]
        mask_off = out_sh
        c2_off = out_sh + kc * nwords

        # resident W_eff slab: [128, kc, out_sh] bf16, preloaded with base-coeff
        w_sb = wpool.tile([P, kc, out_sh], dt.bfloat16)
        nc.sync.dma_start(w_sb[:], bmc[:, :, :])

        # Sacrificial 2D copies: absorb DMA semaphore waits into the DVE's
        # vector clock, so TensorTensor instructions (1 wait slot only) never
        # need to carry a DMA wait on top of a slot wait.
        warm = cpool.tile([P, 2], dt.int32)
        nc.vector.tensor_copy(warm[:, 0:1], consts_sb[:, :1])
        nc.vector.tensor_copy(warm[:, 1:2], w_sb[:, 0, :1].bitcast(dt.int16))

        # ---- phase 1: unpack mask + fold into W_eff (in place) ----
        for k in range(kc):
            # sh = word_{o//32} >> (o%32)
            sh_t = upool.tile([P, out_sh], dt.int32, tag="sh")
            mask_k = consts_sb[:, mask_off + k * nwords : mask_off + (k + 1) * nwords]
            nc.vector.tensor_tensor(
                sh_t[:],
                mask_k[:, :, None].to_broadcast((P, nwords, NBITS)),
                shifts_sb[:],
                Alu.logical_shift_right,
            )
            c2_col = consts_sb[:, c2_off + k : c2_off + k + 1].bitcast(dt.float32)
            d_t = upool.tile([P, out_sh], dt.float32, tag="d")
            if FUSED_TS:
                # d = (sh & 1) * 2c
                nc.vector.tensor_scalar(
                    d_t[:], sh_t[:], 1, c2_col, Alu.bitwise_and, Alu.mult
                )
            else:
                bit_t = upool.tile([P, out_sh], dt.int32, tag="bit")
                nc.vector.tensor_scalar(bit_t[:], sh_t[:], 1, None, Alu.bitwise_and)
                bf_t = upool.tile([P, out_sh], dt.float32, tag="bf")
                nc.vector.tensor_copy(bf_t[:], bit_t[:])
                nc.vector.tensor_scalar(d_t[:], bf_t[:], c2_col, None, Alu.mult)
            # W[k] = (base - c) + d   (in place on the preloaded slab)
            nc.vector.tensor_tensor(w_sb[:, k, :], d_t[:], w_sb[:, k, :], Alu.add)

        # ---- phase 2: stream tokens through the resident W_eff ----
        for t in range(tt):
            x_sb = xpool.tile([P, kc, P], dt.bfloat16, tag="x")
            nc.sync.dma_start(x_sb[:], xt[t])
            ps = pspool.tile([P, out_sh], dt.float32, tag="ps")
            for k in range(kc):
                nc.tensor.matmul(
                    ps[:],
                    lhsT=x_sb[:, k, :],
                    rhs=w_sb[:, k, :],
                    start=(k == 0),
                    stop=(k == kc - 1),
                )
            o_sb = opool.tile([P, out_sh], dt.float32, tag="o")
            nc.vector.tensor_copy(o_sb[:], ps[:])
            nc.sync.dma_start(y[t * P : (t + 1) * P, :], o_sb[:])

    nc.finalize()  # Bacc: reg alloc + event-sem wait splitting
    return nc


def make_in_maps(x, base_t, coeff, mask, in_dim=IN, ntok=NTOK, out_sh=OUT_SH, ncores=NCORES):
    kc = in_dim // P
    tt = ntok // P
    nwords = out_sh // NBITS

    x2d = np.ascontiguousarray(x.reshape(-1, in_dim))
    xT = np.ascontiguousarray(x2d.T).astype(ml_dtypes.bfloat16)  # (in, ntok)
    # (k,p,t,c) -> (t,p,k,c): per token tile, per partition, k-chunks contiguous
    xt_tiled = np.ascontiguousarray(xT.reshape(kc, P, tt, P).transpose(2, 1, 0, 3))

    coeff = coeff.astype(np.float32)
    c2 = np.ascontiguousarray((2.0 * coeff).reshape(kc, P).T)  # (P, kc) f32
    shifts = np.broadcast_to(
        np.tile(np.arange(NBITS, dtype=np.int32), nwords), (P, out_sh)
    )

    bmc_full = base_t.astype(np.float32) - coeff[:, None]  # (in, out)

    in_maps = []
    for j in range(ncores):
        # (kc, P, out_sh) -> (P, kc, out_sh), bf16
        bmc_j = np.ascontiguousarray(
            bmc_full[:, j * out_sh : (j + 1) * out_sh]
            .reshape(kc, P, out_sh)
            .transpose(1, 0, 2)
            .astype(ml_dtypes.bfloat16)
        )
        # mask slab tiled to [p, k*nwords+w]
        m_j = (
            mask[:, j * nwords : (j + 1) * nwords]
            .reshape(kc, P, nwords)
            .transpose(1, 0, 2)
            .reshape(P, kc * nwords)
            .astype(np.int32)
        )
        consts = np.concatenate(
            [shifts, m_j, c2.view(np.int32)], axis=1
        ).astype(np.int32)
        in_maps.append(
            {
                "xt": xt_tiled,
                "bmc": bmc_j,
                "consts": np.ascontiguousarray(consts),
            }
        )
    return in_maps


_CACHED = {}


def kernel(x, base_t, coeff, mask):
    from concourse.bass_utils import run_bass_kernel_spmd

    if "nc" not in _CACHED:
        _CACHED["nc"] = build_bass()
    nc = _CACHED["nc"]

    in_maps = make_in_maps(x, base_t, coeff, mask)
    res = run_bass_kernel_spmd(nc, in_maps, core_ids=list(range(NCORES)))
    outs = res.results
    y = np.concatenate([outs[j]["y"] for j in range(NCORES)], axis=1)
    y = y.reshape(B, S, OUT).astype(np.float32)
    return y


if __name__ == "__main__":
    # smoke test at full size
    rng = np.random.default_rng(0)
    x = rng.standard_normal((B, S, IN), dtype=np.float32)
    base_t = (rng.standard_normal((IN, OUT), dtype=np.float32) * 0.02).astype(np.float32)
    coeff = (rng.random(IN, dtype=np.float32) * 0.01).astype(np.float32)
    mask = rng.integers(0, 2**31 - 1, size=(IN, OUT // NBITS), dtype=np.int32)
    y = kernel(x=x, base_t=base_t, coeff=coeff, mask=mask)
    print("y", y.shape, y.dtype)


# revision 24
# speedup vs baseline: 226.1976x; 226.1976x over previous
"""Trainium2 kernel for nn_BinaryDiffRow.

Math: y = x @ base_t + (x * coeff) @ S,  S = unpack_signs(mask) in {-1,+1}
Fold: y = x @ W_eff,  W_eff = base_t + coeff[:,None] * S   (single matmul)
      W_eff = (base_t - coeff) + 2*coeff*bit,  bit in {0,1}
      (base_t - coeff folded on host; bit unpacked on device)

Sharding (tensor parallel over output columns, 8 cores):
  core j owns output columns [512j, 512j+512).
  - Builds its W_eff slab (4096 x 512, bf16) once on-device:
    bit-unpack of mask via DVE shift/AND, kept resident in SBUF.
  - Streams all 8192 tokens of x (host-pretransposed, bf16) through the PE,
    accumulating psum[128tok, 512] over 32 k-chunks.
  - Host concatenates the 8 column slabs into the full output.
"""

import os
import sys

import numpy as np

for _p in ("/opt/trn_rl_repo",):
    if _p not in sys.path and os.path.isdir(_p):
        sys.path.insert(0, _p)

import ml_dtypes  # noqa: E402

# --- problem constants (hardcoded per contract) ---
B, S, IN, OUT = 4, 2048, 4096, 4096
NTOK = B * S  # 8192
NCORES = 8
OUT_SH = OUT // NCORES  # 512
P = 128
NBITS = 32



def build_bass(in_dim=IN, ntok=NTOK, out_sh=OUT_SH, x_bufs=6, ps_bufs=6, repeat_phase2=1):
    """Build the single-core Bass program (SPMD: all cores run this)."""
    import concourse.mybir as mybir
    import concourse.tile as tile
    from concourse import bacc
    from contextlib import ExitStack

    kc = in_dim // P  # k-chunks
    tt = ntok // P  # token tiles
    nwords = out_sh // NBITS

    # Bacc (not plain Bass): its finalize() runs generate_event_semaphores,
    # which splits multi-sem waits — walrus only allows 1 wait/instruction.
    nc = bacc.Bacc("TRN2")
    dt = mybir.dt
    Alu = mybir.AluOpType

    xt = nc.dram_tensor("xt", (tt, P, kc, P), dt.bfloat16, kind="ExternalInput")
    # host ships (base_t - coeff) pre-tiled to (P, kc, out_sh) in bf16;
    # DMA'd directly into the resident W slab, then the unpacked +/-2c*bit
    # delta is accumulated in place (no per-k DMAs -> no DMA-wait pileups).
    bmc = nc.dram_tensor("bmc", (P, kc, out_sh), dt.bfloat16, kind="ExternalInput")
    # merged int32 const block: [shift table | mask tiled | 2*coeff bits]
    # one DMA -> one semaphore wait for all phase-1 consumers (the 3D-AP
    # TensorTensor encoding only has room for a single sync wait).
    cw = out_sh + kc * nwords + kc
    consts = nc.dram_tensor("consts", (P, cw), dt.int32, kind="ExternalInput")
    y = nc.dram_tensor("y", (ntok, out_sh), dt.float32, kind="ExternalOutput")

    with ExitStack() as ctx:
        tc = ctx.enter_context(tile.TileContext(nc))
        cpool = ctx.enter_context(tc.tile_pool(name="consts", bufs=1))
        wpool = ctx.enter_context(tc.tile_pool(name="w", bufs=1))
        upool = ctx.enter_context(tc.tile_pool(name="unpack", bufs=2))
        xpool = ctx.enter_context(tc.tile_pool(name="x", bufs=x_bufs))
        opool = ctx.enter_context(tc.tile_pool(name="out", bufs=3))
        pspool = ctx.enter_context(tc.tile_pool(name="ps", bufs=ps_bufs, space="PSUM"))

        consts_sb = cpool.tile([P, cw], dt.int32)
        nc.sync.dma_start(consts_sb[:], consts[:, :])
        shifts_sb = consts_sb[:, :out_sh]
        mask_off = out_sh
        c2_off = out_sh + kc * nwords

        # resident W_eff slab: [128, kc, out_sh] bf16, preloaded with base-coeff
        w_sb = wpool.tile([P, kc, out_sh], dt.bfloat16)
        nc.sync.dma_start(w_sb[:], bmc[:, :, :])

        # Sacrificial 2D copies: absorb DMA semaphore waits into the DVE's
        # vector clock, so TensorTensor instructions (1 wait slot only) never
        # need to carry a DMA wait on top of a slot wait.
        warm = cpool.tile([P, 2], dt.int32)
        nc.vector.tensor_copy(warm[:, 0:1], consts_sb[:, :1])
        nc.vector.tensor_copy(warm[:, 1:2], w_sb[:, 0, :1].bitcast(dt.int16))

        # ---- phase 1: unpack mask + fold into W_eff (in place) ----
        for k in range(kc):
            # sh = word_{o//32} >> (o%32)
            sh_t = upool.tile([P, out_sh], dt.int32, tag="sh")
            mask_k = consts_sb[:, mask_off + k * nwords : mask_off + (k + 1) * nwords]
            nc.vector.tensor_tensor(
                sh_t[:],
                mask_k[:, :, None].to_broadcast((P, nwords, NBITS)),
                shifts_sb[:],
                Alu.logical_shift_right,
            )
            c2_col = consts_sb[:, c2_off + k : c2_off + k + 1].bitcast(dt.float32)
            bit_t = upool.tile([P, out_sh], dt.int32, tag="bit")
            nc.vector.tensor_scalar(bit_t[:], sh_t[:], 1, None, Alu.bitwise_and)
            # d = 2c * bit  (ACT engine: scale-multiply with i32->f32 cast,
            # offloads work from the DVE which is the phase-1 bottleneck)
            d_t = upool.tile([P, out_sh], dt.float32, tag="d")
            nc.scalar.activation(
                d_t[:], bit_t[:], mybir.ActivationFunctionType.Copy, scale=c2_col
            )
            # W[k] = (base - c) + d   (in place on the preloaded slab)
            nc.vector.tensor_tensor(w_sb[:, k, :], d_t[:], w_sb[:, k, :], Alu.add)

        # ---- phase 2: stream tokens through the resident W_eff ----
        def phase2():
            for t in range(tt):
                x_sb = xpool.tile([P, kc, P], dt.bfloat16, tag="x")
                nc.sync.dma_start(x_sb[:], xt[t])
                ps = pspool.tile([P, out_sh], dt.float32, tag="ps")
                for k in range(kc):
                    nc.tensor.matmul(
                        ps[:],
                        lhsT=x_sb[:, k, :],
                        rhs=w_sb[:, k, :],
                        start=(k == 0),
                        stop=(k == kc - 1),
                    )
                o_sb = opool.tile([P, out_sh], dt.float32, tag="o")
                nc.vector.tensor_copy(o_sb[:], ps[:])
                nc.sync.dma_start(y[t * P : (t + 1) * P, :], o_sb[:])

        if repeat_phase2 == 1:
            phase2()
        else:
            # benchmarking only: repeat the streaming phase in a HW loop so
            # one NEFF execution amortizes the ~85ms axon dispatch overhead
            with tc.For_i(0, repeat_phase2, 1):
                phase2()

    nc.finalize()  # Bacc: reg alloc + event-sem wait splitting
    return nc


def make_in_maps(x, base_t, coeff, mask, in_dim=IN, ntok=NTOK, out_sh=OUT_SH, ncores=NCORES):
    kc = in_dim // P
    tt = ntok // P
    nwords = out_sh // NBITS

    x2d = np.ascontiguousarray(x.reshape(-1, in_dim))
    xT = np.ascontiguousarray(x2d.T).astype(ml_dtypes.bfloat16)  # (in, ntok)
    # (k,p,t,c) -> (t,p,k,c): per token tile, per partition, k-chunks contiguous
    xt_tiled = np.ascontiguousarray(xT.reshape(kc, P, tt, P).transpose(2, 1, 0, 3))

    coeff = coeff.astype(np.float32)
    c2 = np.ascontiguousarray((2.0 * coeff).reshape(kc, P).T)  # (P, kc) f32
    shifts = np.broadcast_to(
        np.tile(np.arange(NBITS, dtype=np.int32), nwords), (P, out_sh)
    )

    bmc_full = base_t.astype(np.float32) - coeff[:, None]  # (in, out)

    in_maps = []
    for j in range(ncores):
        # (kc, P, out_sh) -> (P, kc, out_sh), bf16
        bmc_j = np.ascontiguousarray(
            bmc_full[:, j * out_sh : (j + 1) * out_sh]
            .reshape(kc, P, out_sh)
            .transpose(1, 0, 2)
            .astype(ml_dtypes.bfloat16)
        )
        # mask slab tiled to [p, k*nwords+w]
        m_j = (
            mask[:, j * nwords : (j + 1) * nwords]
            .reshape(kc, P, nwords)
            .transpose(1, 0, 2)
            .reshape(P, kc * nwords)
            .astype(np.int32)
        )
        consts = np.concatenate(
            [shifts, m_j, c2.view(np.int32)], axis=1
        ).astype(np.int32)
        in_maps.append(
            {
                "xt": xt_tiled,
                "bmc": bmc_j,
                "consts": np.ascontiguousarray(consts),
            }
        )
    return in_maps


_CACHED = {}


def kernel(x, base_t, coeff, mask):
    from concourse.bass_utils import run_bass_kernel_spmd

    if "nc" not in _CACHED:
        _CACHED["nc"] = build_bass()
    nc = _CACHED["nc"]

    x = np.asarray(x, dtype=np.float32)
    base_t = np.asarray(base_t, dtype=np.float32)
    coeff = np.asarray(coeff, dtype=np.float32)
    mask = np.asarray(mask, dtype=np.int32)
    in_maps = make_in_maps(x, base_t, coeff, mask)
    res = run_bass_kernel_spmd(nc, in_maps, core_ids=list(range(NCORES)))
    outs = res.results
    y = np.concatenate([outs[j]["y"] for j in range(NCORES)], axis=1)
    y = y.reshape(B, S, OUT).astype(np.float32)
    return y


if __name__ == "__main__":
    # smoke test at full size
    rng = np.random.default_rng(0)
    x = rng.standard_normal((B, S, IN), dtype=np.float32)
    base_t = (rng.standard_normal((IN, OUT), dtype=np.float32) * 0.02).astype(np.float32)
    coeff = (rng.random(IN, dtype=np.float32) * 0.01).astype(np.float32)
    mask = rng.integers(0, 2**31 - 1, size=(IN, OUT // NBITS), dtype=np.int32)
    y = kernel(x=x, base_t=base_t, coeff=coeff, mask=mask)
    print("y", y.shape, y.dtype)

